# revision 13
# baseline (speedup 1.0000x reference)
"""CoEvoGNN message-passing kernel for 8 TRN2 NeuronCores.

Recurrence (T = t_train + t_forecast = 5 steps, K = 2 fusion depth):
    for t in 1..5:
        z_t = sum_{k=1..min(t,2)} [H_{t-k} | mean_s H_{t-k}[idx[t-k]]] @ W[k-1]
        H_t = l2normalize(relu(z_t))

Key structural facts exploited:
  * The aggregation A_{t'} = mean_s H_{t'}[idx[t']] is shared between step
    t'+1 (with W[0]) and t'+2 (with W[1]) -> only T aggregations total.
  * The 1/S mean is folded into the bottom half of the weights host-side.

Sharding: nodes are split into 8 contiguous shards of 12500. Each core
computes its shard of every H_t; the random-neighbor gather needs the FULL
H_{t-1}, so after each step the shards are AllGathered into a full
[100000, 64] DRAM replica per core; the per-node gather is an SWDGE
indirect DMA (one 256B row descriptor per sampled neighbor).

On-chip layout: nodes on partitions. For the matmul, cat_t = [H_t | A_t]
is kept TRANSPOSED in SBUF (catT: [128 dins, nodes]), produced by PE
transposes, so z = catT.T @ W accumulates over 2 steps in PSUM.
"""

import numpy as np

# ---- problem constants (hardcoded per contract) ----
N = 100000
D = 64
S = 20
T_TRAIN = 4
T_FC = 1
T = T_TRAIN + T_FC
NCORES = 8
NSH = N // NCORES  # 12500
P = 128


def _split_multi_waits(nc, cap=1):
    """The container's walrus build accepts at most `cap` sync-wait commands
    per instruction; Tile emits more (notably on the kernel-tail drain).
    Move excess waits onto dedicated NOPs inserted just before the
    instruction on the same engine — semantically identical (the engine's
    sequencer satisfies the waits in sequence before issuing)."""
    import concourse.mybir as mybir

    cnt = 0
    for f in nc.m.functions:
        for bb in f.blocks:
            insts = list(bb.instructions)
            out = []
            changed = False
            for inst in insts:
                si = inst.sync_info
                if si is not None and si.on_wait is not None and len(si.on_wait) > cap:
                    waits = list(si.on_wait)
                    extra, keep = waits[:-cap], waits[-cap:]
                    for w in extra:
                        cnt += 1
                        out.append(
                            mybir.InstNoOp(
                                name=f"waitsplit-{cnt}",
                                engine=inst.engine,
                                sync_info=mybir.SyncInfo(on_wait=[w], on_update=[]),
                                bass_nofuse=True,
                            )
                        )
                    si.on_wait = keep
                    changed = True
                out.append(inst)
            if changed:
                bb.instructions = out
    return cnt


def build_nc(n=N, d=D, s=S, t_steps=T, ncores=NCORES, gb=2, gbufs=3, debug=False):
    """Build the SPMD Bass program (same NEFF on every core)."""
    import concourse.bass as bass
    import concourse.mybir as mybir
    import concourse.tile as tile
    from concourse.masks import make_identity

    f32 = mybir.dt.float32
    i32 = mybir.dt.int32
    nsh = n // ncores
    ntiles = (nsh + P - 1) // P
    ncols = ntiles * P  # catT column capacity

    nc = bass.Bass(num_devices=ncores)

    Hfull0 = nc.dram_tensor("H0", [n, d], f32, kind="ExternalInput")
    H0sh = nc.dram_tensor("H0sh", [nsh, d], f32, kind="ExternalInput")
    W0 = nc.dram_tensor("W0", [2 * d, d], f32, kind="ExternalInput")
    W1 = nc.dram_tensor("W1", [2 * d, d], f32, kind="ExternalInput")
    nidx = nc.dram_tensor("nidx", [t_steps, nsh, s], i32, kind="ExternalInput")
    Hout = nc.dram_tensor("Hout", [t_steps, nsh, d], f32, kind="ExternalOutput")

    if debug:
        Adbg = nc.dram_tensor("Adbg", [nsh, d], f32, kind="ExternalOutput")
        Gdbg = nc.dram_tensor("Gdbg", [P, s * d], f32, kind="ExternalOutput")
        Idbg = nc.dram_tensor("Idbg", [P, gb * s], i32, kind="ExternalOutput")

    # per-step collective bounce buffers (internal DRAM)
    cc_in = [nc.dram_tensor(f"ccin{t}", [nsh, d], f32) for t in range(1, t_steps)]
    # Shared addr space (direct HBM-HBM gather target) needs >4 ranks
    hf_space = "Shared" if ncores > 4 else "Local"
    hfull = [
        nc.dram_tensor(f"hfull{t}", [n, d], f32, addr_space=hf_space)
        for t in range(1, t_steps)
    ]

    with tile.TileContext(nc) as tc:
        with (
            tc.tile_pool(name="const", bufs=1) as cpool,
            tc.tile_pool(name="cat", bufs=1) as catpool,
            tc.tile_pool(name="g", bufs=gbufs) as gpool,
            tc.tile_pool(name="idx", bufs=3) as ipool,
            tc.tile_pool(name="work", bufs=3) as wpool,
            tc.tile_pool(name="psum", bufs=2, space="PSUM") as ppool,
        ):
            ident = cpool.tile([P, P], f32, tag="ident")
            make_identity(nc, ident[:])
            w0sb = cpool.tile([2 * d, d], f32, tag="w0")
            nc.sync.dma_start(out=w0sb[:], in_=W0[:, :])
            w1sb = cpool.tile([2 * d, d], f32, tag="w1")
            nc.sync.dma_start(out=w1sb[:], in_=W1[:, :])

            # catT[j]: transposed [Ht | At] for source time t' with t' % 3 == j.
            # rows 0:d = H^T, rows d:2d = A^T; columns = node within shard.
            catT = [
                catpool.tile([2 * d, ncols], f32, tag=f"cat{j}", name=f"catT{j}")
                for j in range(3)
            ]

            def tp(i):
                """(row0, p) for tile i."""
                r0 = i * P
                return r0, min(P, nsh - r0)

            # ---- step 0 prologue: H_0^T into catT[0] top half ----
            for i in range(ntiles):
                r0, p = tp(i)
                hin = wpool.tile([P, d], f32, tag="hin")
                nc.sync.dma_start(out=hin[:p], in_=H0sh[r0 : r0 + p, :])
                pth = ppool.tile([P, P], f32, tag="pth")
                nc.tensor.transpose(out=pth[:d, :p], in_=hin[:p, :d], identity=ident[:p, :p])
                nc.scalar.copy(out=catT[0][0:d, r0 : r0 + p], in_=pth[:d, :p])

            # ---- recurrent steps ----
            for t in range(1, t_steps + 1):
                src = Hfull0 if t == 1 else hfull[t - 2]
                bw = catT[(t - 1) % 3]  # cat_{t-1}: A_{t-1}^T written now

                # gather + mean + transpose, in groups of up to `gb` tiles.
                # This walrus's indirect-DMA unroll supports exactly one
                # index per destination partition (span = the partition's
                # contiguous extent), so each sample s is its own gather.
                for g0 in range(0, ntiles, gb):
                    gtiles = min(gb, ntiles - g0)
                    r0 = g0 * P
                    rows = min(gtiles * P, nsh - r0)
                    it = ipool.tile([P, gb * s], i32, tag="it")
                    full_j = rows // P  # how many full 128-row tiles
                    if full_j:
                        nc.sync.dma_start(
                            out=it[:, : full_j * s].rearrange("p (j s) -> p j s", s=s),
                            in_=nidx[t - 1, r0 : r0 + full_j * P, :].rearrange(
                                "(j p) s -> p j s", p=P
                            ),
                        )
                    if rows % P:  # ragged tail tile
                        pr = rows % P
                        nc.vector.memset(it[:, full_j * s : (full_j + 1) * s], 0)
                        nc.sync.dma_start(
                            out=it[:pr, full_j * s : (full_j + 1) * s],
                            in_=nidx[t - 1, r0 + full_j * P : r0 + rows, :],
                        )
                    asum = wpool.tile([P, gb * d], f32, tag="asum")
                    for j in range(gtiles):
                        pj = min(P, nsh - (g0 + j) * P)
                        gt = gpool.tile([P, s * d], f32, tag="g")
                        for smp in range(s):
                            nc.gpsimd.indirect_dma_start(
                                out=gt[:pj, smp * d : (smp + 1) * d],
                                out_offset=None,
                                in_=src[:, :],
                                in_offset=bass.IndirectOffsetOnAxis(
                                    ap=it[:pj, j * s + smp : j * s + smp + 1],
                                    axis=0,
                                ),
                            )
                        nc.vector.tensor_reduce(
                            out=asum[:pj, j * d : (j + 1) * d],
                            in_=gt[:pj, :].rearrange("p (s d) -> p d s", s=s, d=d),
                            axis=mybir.AxisListType.X,
                            op=mybir.AluOpType.add,
                        )
                    if debug and t == 1:
                        if g0 == 0:
                            nc.sync.dma_start(out=Gdbg[:, :], in_=gt[:])
                            nc.sync.dma_start(out=Idbg[:, :], in_=it[:])
                        full_rows = (rows // P) * P
                        if full_rows:
                            nc.sync.dma_start(
                                out=Adbg[r0 : r0 + full_rows, :].rearrange(
                                    "(j p) d -> p j d", p=P
                                ),
                                in_=asum[:, : (rows // P) * d].rearrange(
                                    "p (j d) -> p j d", d=d
                                ),
                            )
                        if rows % P:
                            pr = rows % P
                            nc.sync.dma_start(
                                out=Adbg[r0 + full_rows : r0 + rows, :],
                                in_=asum[
                                    :pr, (rows // P) * d : (rows // P + 1) * d
                                ],
                            )
                    # transpose pairs of tiles: [128 nodes, 2*d] -> [2*d, 128]
                    for q in range(0, gtiles, 2):
                        qt = min(2, gtiles - q)
                        pta = ppool.tile([P, P], f32, tag="pta")
                        nc.tensor.transpose(
                            out=pta[: qt * d, :],
                            in_=asum[:, q * d : (q + qt) * d],
                            identity=ident[:, :],
                        )
                        for u in range(qt):
                            ti = g0 + q + u
                            c0, pc = tp(ti)
                            nc.scalar.copy(
                                out=bw[d : 2 * d, c0 : c0 + pc],
                                in_=pta[u * d : (u + 1) * d, :pc],
                            )

                # per-tile matmul + relu + l2norm
                for i in range(ntiles):
                    r0, p = tp(i)
                    pz = ppool.tile([P, d], f32, tag="pz")
                    nc.tensor.matmul(
                        out=pz[:p, :],
                        lhsT=catT[(t - 1) % 3][:, r0 : r0 + p],
                        rhs=w0sb[:],
                        start=True,
                        stop=(t == 1),
                    )
                    if t >= 2:
                        nc.tensor.matmul(
                            out=pz[:p, :],
                            lhsT=catT[(t - 2) % 3][:, r0 : r0 + p],
                            rhs=w1sb[:],
                            start=False,
                            stop=True,
                        )
                    h = wpool.tile([P, d], f32, tag="h")
                    nc.scalar.activation(
                        out=h[:p], in_=pz[:p, :], func=mybir.ActivationFunctionType.Relu
                    )
                    hsq = wpool.tile([P, d], f32, tag="hsq")
                    ss = wpool.tile([P, 1], f32, tag="ss")
                    nc.scalar.activation(
                        out=hsq[:p],
                        in_=h[:p],
                        func=mybir.ActivationFunctionType.Square,
                        accum_out=ss[:p],
                    )
                    ssc = wpool.tile([P, 1], f32, tag="ssc")
                    # max(||h||, eps) == sqrt(max(ss, eps^2)); eps=1e-12
                    nc.vector.tensor_scalar_max(out=ssc[:p], in0=ss[:p], scalar1=1e-24)
                    rr = wpool.tile([P, 1], f32, tag="rr")
                    nc.vector.reciprocal(out=rr[:p], in_=ssc[:p])
                    rn = wpool.tile([P, 1], f32, tag="rn")
                    nc.scalar.activation(
                        out=rn[:p], in_=rr[:p], func=mybir.ActivationFunctionType.Sqrt
                    )
                    hn = wpool.tile([P, d], f32, tag="hn")
                    nc.vector.tensor_scalar_mul(
                        out=hn[:p], in0=h[:p], scalar1=rn[:p, :1]
                    )
                    nc.sync.dma_start(out=Hout[t - 1, r0 : r0 + p, :], in_=hn[:p])
                    if t < t_steps:
                        nc.sync.dma_start(out=cc_in[t - 1][r0 : r0 + p, :], in_=hn[:p])
                        pth = ppool.tile([P, P], f32, tag="pth")
                        nc.tensor.transpose(
                            out=pth[:d, :p], in_=hn[:p, :d], identity=ident[:p, :p]
                        )
                        nc.scalar.copy(
                            out=catT[t % 3][0:d, r0 : r0 + p], in_=pth[:d, :p]
                        )

                if t < t_steps:
                    nc.gpsimd.collective_compute(
                        "AllGather",
                        mybir.AluOpType.bypass,
                        replica_groups=[list(range(ncores))],
                        ins=[cc_in[t - 1][:, :].opt()],
                        outs=[hfull[t - 1][:, :].opt()],
                    )

    _split_multi_waits(nc)
    return nc


_NC_CACHE = {}


def _execute(H_0, W, neigh_idx, **run_kwargs):
    H_0 = np.ascontiguousarray(np.asarray(H_0), dtype=np.float32)
    W = np.asarray(W)
    neigh_idx = np.ascontiguousarray(np.asarray(neigh_idx), dtype=np.int32)
    assert H_0.shape == (N, D) and W.shape == (2, 2 * D, D)
    assert neigh_idx.shape == (T, N, S)

    # fold the 1/S mean into the aggregation half of each weight matrix
    Wm = np.array(W, dtype=np.float32, copy=True)
    Wm[:, D:, :] /= np.float32(S)
    W0 = np.ascontiguousarray(Wm[0])
    W1 = np.ascontiguousarray(Wm[1])

    from concourse.bass_utils import run_bass_kernel_spmd

    key = "full"
    if key not in _NC_CACHE:
        _NC_CACHE[key] = build_nc()
    nc = _NC_CACHE[key]

    in_maps = []
    for c in range(NCORES):
        lo, hi = c * NSH, (c + 1) * NSH
        in_maps.append(
            {
                "H0": H_0,
                "H0sh": np.ascontiguousarray(H_0[lo:hi]),
                "W0": W0,
                "W1": W1,
                "nidx": np.ascontiguousarray(neigh_idx[:, lo:hi, :]),
            }
        )

    res = run_bass_kernel_spmd(nc, in_maps, core_ids=list(range(NCORES)), **run_kwargs)
    H_all = np.empty((T, N, D), dtype=np.float32)
    for c in range(NCORES):
        H_all[:, c * NSH : (c + 1) * NSH, :] = res.results[c]["Hout"]
    return (H_all[:T_TRAIN], H_all[T_TRAIN:]), res


def kernel(H_0, W, neigh_idx, t_train, t_forecast):
    assert int(t_train) == T_TRAIN and int(t_forecast) == T_FC
    out, _ = _execute(H_0, W, neigh_idx)
    return out


# revision 15
# speedup vs baseline: 1.0078x; 1.0078x over previous
"""CoEvoGNN message-passing kernel for 8 TRN2 NeuronCores.

Recurrence (T = t_train + t_forecast = 5 steps, K = 2 fusion depth):
    for t in 1..5:
        z_t = sum_{k=1..min(t,2)} [H_{t-k} | mean_s H_{t-k}[idx[t-k]]] @ W[k-1]
        H_t = l2normalize(relu(z_t))

Key structural facts exploited:
  * The aggregation A_{t'} = mean_s H_{t'}[idx[t']] is shared between step
    t'+1 (with W[0]) and t'+2 (with W[1]) -> only T aggregations total.
  * The 1/S mean is folded into the bottom half of the weights host-side.

Sharding: nodes are split into 8 contiguous shards of 12500. Each core
computes its shard of every H_t; the random-neighbor gather needs the FULL
H_{t-1}, so after each step the shards are AllGathered into a full
[100000, 64] DRAM replica per core; the per-node gather is an SWDGE
indirect DMA (one 256B row descriptor per sampled neighbor).

On-chip layout: nodes on partitions. For the matmul, cat_t = [H_t | A_t]
is kept TRANSPOSED in SBUF (catT: [128 dins, nodes]), produced by PE
transposes, so z = catT.T @ W accumulates over 2 steps in PSUM.
"""

import numpy as np

# ---- problem constants (hardcoded per contract) ----
N = 100000
D = 64
S = 20
T_TRAIN = 4
T_FC = 1
T = T_TRAIN + T_FC
NCORES = 8
NSH = N // NCORES  # 12500
P = 128


def _split_multi_waits(nc, cap=1):
    """The container's walrus build accepts at most `cap` sync-wait commands
    per instruction; Tile emits more (notably on the kernel-tail drain).
    Move excess waits onto dedicated NOPs inserted just before the
    instruction on the same engine — semantically identical (the engine's
    sequencer satisfies the waits in sequence before issuing)."""
    import concourse.mybir as mybir

    cnt = 0
    for f in nc.m.functions:
        for bb in f.blocks:
            insts = list(bb.instructions)
            out = []
            changed = False
            for inst in insts:
                si = inst.sync_info
                if si is not None and si.on_wait is not None and len(si.on_wait) > cap:
                    waits = list(si.on_wait)
                    extra, keep = waits[:-cap], waits[-cap:]
                    for w in extra:
                        cnt += 1
                        out.append(
                            mybir.InstNoOp(
                                name=f"waitsplit-{cnt}",
                                engine=inst.engine,
                                sync_info=mybir.SyncInfo(on_wait=[w], on_update=[]),
                                bass_nofuse=True,
                            )
                        )
                    si.on_wait = keep
                    changed = True
                out.append(inst)
            if changed:
                bb.instructions = out
    return cnt


def build_nc(n=N, d=D, s=S, t_steps=T, ncores=NCORES, gb=2, gbufs=4, debug=False):
    """Build the SPMD Bass program (same NEFF on every core)."""
    import concourse.bass as bass
    import concourse.mybir as mybir
    import concourse.tile as tile
    from concourse.masks import make_identity

    f32 = mybir.dt.float32
    i32 = mybir.dt.int32
    nsh = n // ncores
    ntiles = (nsh + P - 1) // P
    ncols = ntiles * P  # catT column capacity

    nc = bass.Bass(num_devices=ncores)

    Hfull0 = nc.dram_tensor("H0", [n, d], f32, kind="ExternalInput")
    H0sh = nc.dram_tensor("H0sh", [nsh, d], f32, kind="ExternalInput")
    W0 = nc.dram_tensor("W0", [2 * d, d], f32, kind="ExternalInput")
    W1 = nc.dram_tensor("W1", [2 * d, d], f32, kind="ExternalInput")
    nidx = nc.dram_tensor("nidx", [t_steps, nsh, s], i32, kind="ExternalInput")
    Hout = nc.dram_tensor("Hout", [t_steps, nsh, d], f32, kind="ExternalOutput")

    if debug:
        Adbg = nc.dram_tensor("Adbg", [nsh, d], f32, kind="ExternalOutput")
        Gdbg = nc.dram_tensor("Gdbg", [P, s * d], f32, kind="ExternalOutput")
        Idbg = nc.dram_tensor("Idbg", [P, gb * s], i32, kind="ExternalOutput")

    # per-step collective bounce buffers (internal DRAM)
    cc_in = [nc.dram_tensor(f"ccin{t}", [nsh, d], f32) for t in range(1, t_steps)]
    # Shared addr space (direct HBM-HBM gather target) needs >4 ranks
    hf_space = "Shared" if ncores > 4 else "Local"
    hfull = [
        nc.dram_tensor(f"hfull{t}", [n, d], f32, addr_space=hf_space)
        for t in range(1, t_steps)
    ]

    with tile.TileContext(nc) as tc:
        with (
            tc.tile_pool(name="const", bufs=1) as cpool,
            tc.tile_pool(name="cat", bufs=1) as catpool,
            tc.tile_pool(name="g", bufs=gbufs) as gpool,
            tc.tile_pool(name="idx", bufs=12) as ipool,
            tc.tile_pool(name="work", bufs=4) as wpool,
            tc.tile_pool(name="psum", bufs=2, space="PSUM") as ppool,
        ):
            ident = cpool.tile([P, P], f32, tag="ident")
            make_identity(nc, ident[:])
            w0sb = cpool.tile([2 * d, d], f32, tag="w0")
            nc.sync.dma_start(out=w0sb[:], in_=W0[:, :])
            w1sb = cpool.tile([2 * d, d], f32, tag="w1")
            nc.sync.dma_start(out=w1sb[:], in_=W1[:, :])

            # catT[j]: transposed [Ht | At] for source time t' with t' % 3 == j.
            # rows 0:d = H^T, rows d:2d = A^T; columns = node within shard.
            catT = [
                catpool.tile([2 * d, ncols], f32, tag=f"cat{j}", name=f"catT{j}")
                for j in range(3)
            ]

            def tp(i):
                """(row0, p) for tile i."""
                r0 = i * P
                return r0, min(P, nsh - r0)

            # ---- step 0 prologue: H_0^T into catT[0] top half ----
            for i in range(ntiles):
                r0, p = tp(i)
                hin = wpool.tile([P, d], f32, tag="hin")
                nc.sync.dma_start(out=hin[:p], in_=H0sh[r0 : r0 + p, :])
                pth = ppool.tile([P, P], f32, tag="pth")
                nc.tensor.transpose(out=pth[:d, :p], in_=hin[:p, :d], identity=ident[:p, :p])
                nc.scalar.copy(out=catT[0][0:d, r0 : r0 + p], in_=pth[:d, :p])

            # ---- recurrent steps ----
            for t in range(1, t_steps + 1):
                src = Hfull0 if t == 1 else hfull[t - 2]
                bw = catT[(t - 1) % 3]  # cat_{t-1}: A_{t-1}^T written now

                # gather + mean + transpose, in groups of up to `gb` tiles.
                # This walrus's indirect-DMA unroll supports exactly one
                # index per destination partition (span = the partition's
                # contiguous extent), so each sample s is its own gather.
                for g0 in range(0, ntiles, gb):
                    gtiles = min(gb, ntiles - g0)
                    r0 = g0 * P
                    rows = min(gtiles * P, nsh - r0)
                    it = ipool.tile([P, gb * s], i32, tag="it")
                    full_j = rows // P  # how many full 128-row tiles
                    if full_j:
                        nc.sync.dma_start(
                            out=it[:, : full_j * s].rearrange("p (j s) -> p j s", s=s),
                            in_=nidx[t - 1, r0 : r0 + full_j * P, :].rearrange(
                                "(j p) s -> p j s", p=P
                            ),
                        )
                    if rows % P:  # ragged tail tile
                        pr = rows % P
                        nc.vector.memset(it[:, full_j * s : (full_j + 1) * s], 0)
                        nc.sync.dma_start(
                            out=it[:pr, full_j * s : (full_j + 1) * s],
                            in_=nidx[t - 1, r0 + full_j * P : r0 + rows, :],
                        )
                    asum = wpool.tile([P, gb * d], f32, tag="asum")
                    for j in range(gtiles):
                        pj = min(P, nsh - (g0 + j) * P)
                        gt = gpool.tile([P, s * d], f32, tag="g")
                        for smp in range(s):
                            nc.gpsimd.indirect_dma_start(
                                out=gt[:pj, smp * d : (smp + 1) * d],
                                out_offset=None,
                                in_=src[:, :],
                                in_offset=bass.IndirectOffsetOnAxis(
                                    ap=it[:pj, j * s + smp : j * s + smp + 1],
                                    axis=0,
                                ),
                            )
                        nc.vector.tensor_reduce(
                            out=asum[:pj, j * d : (j + 1) * d],
                            in_=gt[:pj, :].rearrange("p (s d) -> p d s", s=s, d=d),
                            axis=mybir.AxisListType.X,
                            op=mybir.AluOpType.add,
                        )
                    if debug and t == 1:
                        if g0 == 0:
                            nc.sync.dma_start(out=Gdbg[:, :], in_=gt[:])
                            nc.sync.dma_start(out=Idbg[:, :], in_=it[:])
                        full_rows = (rows // P) * P
                        if full_rows:
                            nc.sync.dma_start(
                                out=Adbg[r0 : r0 + full_rows, :].rearrange(
                                    "(j p) d -> p j d", p=P
                                ),
                                in_=asum[:, : (rows // P) * d].rearrange(
                                    "p (j d) -> p j d", d=d
                                ),
                            )
                        if rows % P:
                            pr = rows % P
                            nc.sync.dma_start(
                                out=Adbg[r0 + full_rows : r0 + rows, :],
                                in_=asum[
                                    :pr, (rows // P) * d : (rows // P + 1) * d
                                ],
                            )
                    # transpose pairs of tiles: [128 nodes, 2*d] -> [2*d, 128]
                    for q in range(0, gtiles, 2):
                        qt = min(2, gtiles - q)
                        pta = ppool.tile([P, P], f32, tag="pta")
                        nc.tensor.transpose(
                            out=pta[: qt * d, :],
                            in_=asum[:, q * d : (q + qt) * d],
                            identity=ident[:, :],
                        )
                        for u in range(qt):
                            ti = g0 + q + u
                            c0, pc = tp(ti)
                            nc.scalar.copy(
                                out=bw[d : 2 * d, c0 : c0 + pc],
                                in_=pta[u * d : (u + 1) * d, :pc],
                            )

                # per-tile matmul + relu + l2norm
                for i in range(ntiles):
                    r0, p = tp(i)
                    pz = ppool.tile([P, d], f32, tag="pz")
                    nc.tensor.matmul(
                        out=pz[:p, :],
                        lhsT=catT[(t - 1) % 3][:, r0 : r0 + p],
                        rhs=w0sb[:],
                        start=True,
                        stop=(t == 1),
                    )
                    if t >= 2:
                        nc.tensor.matmul(
                            out=pz[:p, :],
                            lhsT=catT[(t - 2) % 3][:, r0 : r0 + p],
                            rhs=w1sb[:],
                            start=False,
                            stop=True,
                        )
                    h = wpool.tile([P, d], f32, tag="h")
                    nc.scalar.activation(
                        out=h[:p], in_=pz[:p, :], func=mybir.ActivationFunctionType.Relu
                    )
                    hsq = wpool.tile([P, d], f32, tag="hsq")
                    ss = wpool.tile([P, 1], f32, tag="ss")
                    nc.scalar.activation(
                        out=hsq[:p],
                        in_=h[:p],
                        func=mybir.ActivationFunctionType.Square,
                        accum_out=ss[:p],
                    )
                    ssc = wpool.tile([P, 1], f32, tag="ssc")
                    # max(||h||, eps) == sqrt(max(ss, eps^2)); eps=1e-12
                    nc.vector.tensor_scalar_max(out=ssc[:p], in0=ss[:p], scalar1=1e-24)
                    rr = wpool.tile([P, 1], f32, tag="rr")
                    nc.vector.reciprocal(out=rr[:p], in_=ssc[:p])
                    rn = wpool.tile([P, 1], f32, tag="rn")
                    nc.scalar.activation(
                        out=rn[:p], in_=rr[:p], func=mybir.ActivationFunctionType.Sqrt
                    )
                    hn = wpool.tile([P, d], f32, tag="hn")
                    nc.vector.tensor_scalar_mul(
                        out=hn[:p], in0=h[:p], scalar1=rn[:p, :1]
                    )
                    nc.sync.dma_start(out=Hout[t - 1, r0 : r0 + p, :], in_=hn[:p])
                    if t < t_steps:
                        nc.sync.dma_start(out=cc_in[t - 1][r0 : r0 + p, :], in_=hn[:p])
                        pth = ppool.tile([P, P], f32, tag="pth")
                        nc.tensor.transpose(
                            out=pth[:d, :p], in_=hn[:p, :d], identity=ident[:p, :p]
                        )
                        nc.scalar.copy(
                            out=catT[t % 3][0:d, r0 : r0 + p], in_=pth[:d, :p]
                        )

                if t < t_steps:
                    nc.gpsimd.collective_compute(
                        "AllGather",
                        mybir.AluOpType.bypass,
                        replica_groups=[list(range(ncores))],
                        ins=[cc_in[t - 1][:, :].opt()],
                        outs=[hfull[t - 1][:, :].opt()],
                    )

    _split_multi_waits(nc)
    return nc


_NC_CACHE = {}


def _execute(H_0, W, neigh_idx, **run_kwargs):
    H_0 = np.ascontiguousarray(np.asarray(H_0), dtype=np.float32)
    W = np.asarray(W)
    neigh_idx = np.ascontiguousarray(np.asarray(neigh_idx), dtype=np.int32)
    assert H_0.shape == (N, D) and W.shape == (2, 2 * D, D)
    assert neigh_idx.shape == (T, N, S)

    # fold the 1/S mean into the aggregation half of each weight matrix
    Wm = np.array(W, dtype=np.float32, copy=True)
    Wm[:, D:, :] /= np.float32(S)
    W0 = np.ascontiguousarray(Wm[0])
    W1 = np.ascontiguousarray(Wm[1])

    from concourse.bass_utils import run_bass_kernel_spmd

    key = "full"
    if key not in _NC_CACHE:
        _NC_CACHE[key] = build_nc()
    nc = _NC_CACHE[key]

    in_maps = []
    for c in range(NCORES):
        lo, hi = c * NSH, (c + 1) * NSH
        in_maps.append(
            {
                "H0": H_0,
                "H0sh": np.ascontiguousarray(H_0[lo:hi]),
                "W0": W0,
                "W1": W1,
                "nidx": np.ascontiguousarray(neigh_idx[:, lo:hi, :]),
            }
        )

    res = run_bass_kernel_spmd(nc, in_maps, core_ids=list(range(NCORES)), **run_kwargs)
    H_all = np.empty((T, N, D), dtype=np.float32)
    for c in range(NCORES):
        H_all[:, c * NSH : (c + 1) * NSH, :] = res.results[c]["Hout"]
    return (H_all[:T_TRAIN], H_all[T_TRAIN:]), res


def kernel(H_0, W, neigh_idx, t_train, t_forecast):
    assert int(t_train) == T_TRAIN and int(t_forecast) == T_FC
    out, _ = _execute(H_0, W, neigh_idx)
    return out
